# revision 41
# baseline (speedup 1.0000x reference)
"""MultiHeadDecoder (moe_routing) Trainium2 kernel.

Strategy: expert-parallel. Each of the 8 cores owns one head's weights.
Host groups samples by head index, pads each group to a common capacity C
(multiple of 16, >= 512), and transposes X so the contraction dim lands on
partitions. Each core runs a dense 2-layer MLP (256->512 relu, 512->2048)
for its head's samples. The kernel returns out^T [OUT_F, C]; the host
transposes and scatters rows back to original order.

Layer 1 computes H^T (hid on partitions):
  H^T[hc] = W1[:, hc].T @ X^T          (lhsT=W1 chunk, rhs=X^T chunk)
Layer 2 keeps out-features on partitions and streams samples:
  out^T[of] = W2[:, of].T @ H^T        (lhsT=W2 [hid,of] tile, rhs=H^T)
so the per-matmul cost scales with the true sample count (split into two
~C/2 column groups to fit PSUM banks) instead of paying full 512-column
matmuls for a mostly-empty tail sample tile. The b2 bias is per-partition
in this layout, so a DVE tensor_scalar_add doubles as the PSUM->SBUF move.

All matmul operands are float16: full PE rate at any free size, FWL weight
loads, half the HBM traffic of f32, and lower PE power (less HAM duty
throttling). PSUM accumulates in f32.

Startup choreography (the original baseline lost ~7us here):
 - The PE is promoted to full clock only after ~6us of gap-free activity,
   so dummy matmuls run from the earliest possible moment (phase 1 reads a
   framework const AP that is memset during the preamble) until xin lands.
 - A dummy activation pulls ACT_TABLE_LOAD (~1.3us) off the critical path.
 - b1/b2 are folded into the xin tails: separate tiny-element DMAs have
   brutal per-descriptor cost and gated stage A by ~2us.
 - Inputs stream on both HWDGE rings in parallel (sync: xin k0 + W2
   c0/c2; scalar: xin k1 + W2 c1/c3). Output stores alternate rings, and
   the final of-tile's add+store is split across Scalar/DVE in small
   chunks to shorten the tail chain (add -> trigger -> transfer -> ack).
"""

import numpy as np

import concourse.bass as bass
import concourse.mybir as mybir
from concourse import bacc
from concourse.tile import TileContext
from concourse.bass_utils import run_bass_kernel_spmd

IN_F, HID, OUT_F, N_HEADS, BATCH = 256, 512, 2048, 8, 4096
N_CORES = 8
P = 128
KI = IN_F // P     # 2  input-feature chunks
HC = HID // P      # 4  hidden chunks
OC = OUT_F // 512  # 4  W2 dma chunks of 512 out-features
OFT = OUT_F // P   # 16 out-feature tiles

f32 = mybir.dt.float32
f16 = mybir.dt.float16
bf16 = mybir.dt.bfloat16

N_WARM0 = 12   # tiny matmuls on the framework's const AP (no memset dep)
N_WARM = 52    # 64-col dummies bridging until xin lands (~10.6us)

_NC_CACHE: dict = {}


def _sgroups(C: int):
    """Split C sample columns into near-equal <=512-wide groups (each a
    multiple of 16, except possibly the last) for PSUM banks."""
    n = -(-C // 512)
    g = ((-(-C // n) + 15) // 16) * 16
    out, s = [], 0
    while s < C:
        sn = min(g, C - s)
        out.append((s, sn))
        s += sn
    return out


def build_nc(C: int):
    """Build the per-core Bass program for capacity C (mult of 16, >=512)."""
    # xin[k] free layout: [ X^T (C) | W1 k-part (HID) | bias (16) ]
    # k=0 bias cols: b1 (HC=4 used); k=1 bias cols: b2 per of-tile (16).
    KF = C + HID + 16
    sgroups = _sgroups(C)

    nc = bacc.Bacc("TRN2", target_bir_lowering=False, debug=False,
                   num_devices=N_CORES)
    # The two xin k-parts ride different HWDGE rings and land together.
    # (One big DMA per part: per-DMA completion latency is ~3.5us, so
    # splitting into smaller transfers only delays the later pieces.)
    xk0 = nc.dram_tensor("xk0", [P, KF], f16, kind="ExternalInput")
    xk1 = nc.dram_tensor("xk1", [P, KF], f16, kind="ExternalInput")
    w2p = nc.dram_tensor("w2p", [OC, P, HC * 512], f16, kind="ExternalInput")
    out_t = nc.dram_tensor("out_t", [OUT_F, C], f32, kind="ExternalOutput")

    relu = mybir.ActivationFunctionType.Relu

    with TileContext(nc) as tc:
        with (
            tc.tile_pool(name="const", bufs=1) as const,
            tc.tile_pool(name="psum", bufs=8, space="PSUM") as psum,
            tc.tile_pool(name="outp", bufs=6) as outp,
        ):
            # Warm tile on gpsimd (otherwise idle) so the sync/scalar rings
            # can start their DMA triggers immediately.
            wsrc = const.tile([P, 64], bf16, tag="warm")
            nc.gpsimd.memset(wsrc[:], 0.0)

            x0 = const.tile([P, KF], f16, tag="x0")
            x1 = const.tile([P, KF], f16, tag="x1")
            w2_cs = [const.tile([P, HC * 512], f16, tag=f"w2_{oc}",
                                name=f"w2_{oc}") for oc in range(OC)]
            # Sync ring: xin k0, then W2 c0/c2 (balanced with scalar).
            nc.sync.dma_start(x0[:], xk0[:])
            nc.sync.dma_start(w2_cs[0][:], w2p[0])
            nc.sync.dma_start(w2_cs[2][:], w2p[2])
            # Scalar ring: xin k1, act-table preload, then W2 c1/c3.
            nc.scalar.dma_start(x1[:], xk1[:])
            wact = const.tile([P, 8], f32, tag="wact")
            nc.scalar.activation(wact[:], wsrc[:, :8], relu)
            nc.scalar.dma_start(w2_cs[1][:], w2p[1])
            nc.scalar.dma_start(w2_cs[3][:], w2p[3])

            b1_ap = x0[:, C + HID: C + HID + HC]             # [P, 4] f16
            # tensor_scalar wants an f32 scalar operand; widen the f16 b2
            # tail once on the otherwise-idle gpsimd engine.
            b2_s = const.tile([P, OFT], f32, tag="b2s")
            nc.gpsimd.tensor_copy(b2_s[:], x1[:, C + HID: C + HID + OFT])
            b2_ap = b2_s

            # per-k (lhsT source, rhs source, lhsT col offset) for stage A
            a_ops = [(x0, x0, C), (x1, x1, C)]

            # HAM warmup: the PE is promoted to full clock only after ~6us
            # of gap-free activity, so keep it busy from the earliest
            # possible moment until xin lands. Phase 1 uses the framework's
            # preamble-memset const AP (no dependency on our own memset);
            # phase 2 uses the bf16 warm tile. All PSUM tiles share one
            # 8-buffer ring (one bank each), so stage B can run several
            # of-tiles ahead of the DVE drain.
            cone = nc.const_aps.aps[(bf16, 1.0)]
            wps = psum.tile([P, 512], f32, tag="ps", name="pswarm")
            for _ in range(N_WARM0):
                nc.tensor.matmul(wps[:1, :1], lhsT=cone[:, :1],
                                 rhs=cone[:, :1], start=True, stop=True)
            for _ in range(N_WARM):
                nc.tensor.matmul(wps[:64, :64], lhsT=wsrc[:, :64],
                                 rhs=wsrc[:, :64], start=True, stop=True)

            # Stage A: H^T [hid(part), sample(free)], relu(x @ W1 + b1).
            # k-outer so the k0 matmuls (whose operands land first) run
            # while xin k1 is still in flight.
            ht = const.tile([P, HC, C], f16)
            for (s0, sn) in sgroups:
                pss = [psum.tile([P, 512], f32, tag="ps", name="psA")
                       for _ in range(HC)]
                for k, (lsrc, rsrc, lo) in enumerate(a_ops):
                    for hc in range(HC):
                        nc.tensor.matmul(
                            pss[hc][:, :sn],
                            lhsT=lsrc[:, lo + hc * P: lo + (hc + 1) * P],
                            rhs=rsrc[:, s0:s0 + sn],
                            start=(k == 0), stop=(k == KI - 1),
                        )
                for hc in range(HC):
                    nc.scalar.activation(
                        ht[:, hc, s0:s0 + sn], pss[hc][:, :sn], relu,
                        bias=b1_ap[:, hc:hc + 1],
                    )

            # Stage B: out^T[of-tile] = sum_hc W2[hc,of].T @ H^T[hc] + b2.
            # Samples are the moving dim, so cost tracks C exactly; the
            # per-partition b2 rides the DVE PSUM->SBUF move.
            ndma = 0
            for of in range(OFT):
                oc, o0 = of // (OFT // OC), (of % (OFT // OC)) * P
                pss = []
                for gi, (s0, sn) in enumerate(sgroups):
                    ps = psum.tile([P, 512], f32, tag="ps", name="psB")
                    pss.append(ps)
                    for hc in range(HC):
                        nc.tensor.matmul(
                            ps[:, :sn],
                            lhsT=w2_cs[oc][:, hc * 512 + o0:
                                           hc * 512 + o0 + P],
                            rhs=ht[:, hc, s0:s0 + sn],
                            start=(hc == 0), stop=(hc == HC - 1),
                        )
                ot = outp.tile([P, C], f32, tag="ot")
                if of < OFT - 1:
                    for gi, (s0, sn) in enumerate(sgroups):
                        nc.vector.tensor_scalar_add(
                            out=ot[:, s0:s0 + sn],
                            in0=pss[gi][:, :sn],
                            scalar1=b2_ap[:, of:of + 1],
                        )
                    eng = nc.scalar if ndma % 2 == 0 else nc.sync
                    eng.dma_start(out_t[of * P:(of + 1) * P, :], ot[:, :C])
                    ndma += 1
                else:
                    # Final of-tile: one chunk per sample group, each with
                    # its own add engine and its own trigger ring, so the
                    # two add->trigger->transfer chains run fully parallel
                    # (more chunks just serialize 590ns triggers per ring).
                    ident = mybir.ActivationFunctionType.Identity
                    for gi, (s0, sn) in enumerate(sgroups):
                        if gi % 2 == 0:
                            nc.scalar.activation(
                                ot[:, s0:s0 + sn], pss[gi][:, :sn],
                                ident, bias=b2_ap[:, of:of + 1],
                            )
                            nc.sync.dma_start(
                                out_t[of * P:(of + 1) * P, s0:s0 + sn],
                                ot[:, s0:s0 + sn])
                        else:
                            nc.vector.tensor_scalar_add(
                                out=ot[:, s0:s0 + sn],
                                in0=pss[gi][:, :sn],
                                scalar1=b2_ap[:, of:of + 1],
                            )
                            nc.scalar.dma_start(
                                out_t[of * P:(of + 1) * P, s0:s0 + sn],
                                ot[:, s0:s0 + sn])
                        ndma += 1

    nc.compile()
    return nc


def kernel(X, X_head_idx, W1, b1, W2, b2):
    X = np.ascontiguousarray(np.asarray(X, dtype=np.float32))
    idx = np.asarray(X_head_idx).astype(np.int64)
    W1 = np.asarray(W1, dtype=np.float32)
    b1 = np.asarray(b1, dtype=np.float32)
    W2 = np.asarray(W2, dtype=np.float32)
    b2 = np.asarray(b2, dtype=np.float32)

    batch = X.shape[0]
    counts = np.bincount(idx, minlength=N_HEADS)
    order = np.argsort(idx, kind="stable")
    positions = np.split(order, np.cumsum(counts)[:-1])

    C = max(512, int(-(-counts.max() // 16)) * 16)
    if C not in _NC_CACHE:
        _NC_CACHE[C] = build_nc(C)
    nc = _NC_CACHE[C]

    in_maps = []
    for h in range(N_HEADS):
        pos = positions[h]
        # xin k-parts: [ X^T (C, padded) | W1 k-part | bias tail ]
        # k0 tail: b1 (4 cols); k1 tail: b2 per of-tile (16 cols).
        xin = np.zeros((KI, P, C + HID + 16), dtype=np.float16)
        if len(pos):
            xin[:, :, :len(pos)] = X[pos].T.reshape(KI, P, len(pos))
        xin[:, :, C:C + HID] = W1[h].reshape(KI, P, HID)
        xin[0, :, C + HID:C + HID + HC] = b1[h].reshape(HC, P).T
        xin[1, :, C + HID:C + HID + OFT] = b2[h].reshape(OFT, P).T
        # w2 packed: [oc, p, hc*512 + o'] = W2[h, hc*128 + p, oc*512 + o']
        w2t = np.transpose(W2[h].reshape(HC, P, OUT_F), (1, 0, 2))  # [p,hc,of]
        w2p = np.empty((OC, P, HC * 512), dtype=np.float16)
        for oc in range(OC):
            w2p[oc] = w2t[:, :, oc * 512:(oc + 1) * 512].reshape(P, HC * 512)
        in_maps.append({"xk0": np.ascontiguousarray(xin[0]),
                        "xk1": np.ascontiguousarray(xin[1]), "w2p": w2p})

    try:
        res = run_bass_kernel_spmd(nc, in_maps, list(range(N_CORES)))
    except Exception:
        res = run_bass_kernel_spmd(nc, in_maps, list(range(N_CORES)))

    out = np.empty((batch, OUT_F), dtype=np.float32)
    for h in range(N_HEADS):
        pos = positions[h]
        if len(pos):
            out[pos] = res.results[h]["out_t"][:, :len(pos)].T
    return out
